# revision 7
# baseline (speedup 1.0000x reference)
"""Batched Procrustes (Kabsch) on 8 Trainium2 NeuronCores.

Strategy (batch-contiguous sharding, fully local per core):
  - 2048 segments -> 8 cores x 256 segments; each core's segments are packed
    segment-per-partition: SBUF tile [128 partitions, 2 halves * SL], where
    partition p of half h holds all points of local segment h*128+p, padded
    with zeros to SL points.
  - Kernel 1 (device): per-segment sums of (sx,sy,sz), (tx,ty,tz) and the 9
    cross products s_i*t_j.  Products+reduction fused in one VectorE
    scalar_tensor_tensor (accum_out) pass per product stream; raw sums on
    ScalarE activation accum_out.  15 sums x 256 segments per core.
  - Host: assemble H = cross - n mu_s mu_t^T, 3x3 SVD in float64, Kabsch
    R = V diag(1,1,d) U^T, t = mu_t - R mu_s  (2048 tiny SVDs, ~10 ms).
  - Kernel 2 (device): aligned = R[seg] @ src + t[seg] with per-partition
    scalars: ScalarE affine (R_i0*sx + t_i) then two VectorE FMA
    scalar_tensor_tensor chained adds.
  - Host: unpack to original point order; return (aligned, (R, t)).
"""

import numpy as np

import concourse.tile as tile
from concourse import bacc, mybir
from concourse.bass_utils import run_bass_kernel_spmd

N = 4194304
B = 2048
NCORES = 8
SPC = B // NCORES          # segments per core (256)
HALVES = 2
ROWS = 128
DEFAULT_SL = 2304          # per-segment padded length (max count <= SL)

F32 = mybir.dt.float32

_cache = {}
LAST_PROFILE = {}


def _build_k1(SL, repeat=1):
    W = HALVES * SL
    nc = bacc.Bacc("TRN2", target_bir_lowering=False, debug=False,
                   num_devices=NCORES)
    ins = {}
    for name in ("sx", "sy", "sz", "tx", "ty", "tz"):
        ins[name] = nc.declare_dram_parameter(name, [ROWS, W], F32,
                                              isOutput=False)
    sums_dve_d = nc.declare_dram_parameter("sums_dve", [ROWS, 18], F32,
                                           isOutput=True)
    sums_act_d = nc.declare_dram_parameter("sums_act", [ROWS, 12], F32,
                                           isOutput=True)

    with tile.TileContext(nc) as tc:
        with (
            tc.tile_pool(name="data", bufs=2) as data,
            tc.tile_pool(name="small", bufs=1) as small,
        ):
            sums_dve = small.tile([ROWS, 18], F32, tag="sums_dve")
            sums_act = small.tile([ROWS, 12], F32, tag="sums_act")
            trash_dve = small.tile([ROWS, SL], F32, tag="trash_dve")
            trash_act = small.tile([ROWS, SL], F32, tag="trash_act")

            for _ in range(repeat):
                for h in range(HALVES):
                    t_in = {}
                    for name in ("sx", "tx", "sy", "ty", "sz", "tz"):
                        t_in[name] = data.tile([ROWS, SL], F32,
                                               name=f"{name}{h}",
                                               tag=f"in_{name}")
                        nc.sync.dma_start(out=t_in[name],
                                          in_=ins[name][:, h * SL:(h + 1) * SL])
                    # ScalarE: 6 raw sums per half
                    for k, name in enumerate(
                            ("sx", "sy", "sz", "tx", "ty", "tz")):
                        nc.scalar.activation(
                            out=trash_act,
                            in_=t_in[name],
                            func=mybir.ActivationFunctionType.Copy,
                            accum_out=sums_act[:, h * 6 + k: h * 6 + k + 1],
                        )
                    # VectorE: 9 fused product+sum per half
                    for i, sn in enumerate(("sx", "sy", "sz")):
                        for j, tn in enumerate(("tx", "ty", "tz")):
                            col = h * 9 + 3 * i + j
                            nc.vector.scalar_tensor_tensor(
                                out=trash_dve,
                                in0=t_in[sn],
                                scalar=1.0,
                                in1=t_in[tn],
                                op0=mybir.AluOpType.mult,
                                op1=mybir.AluOpType.mult,
                                accum_out=sums_dve[:, col:col + 1],
                            )

            nc.sync.dma_start(out=sums_dve_d[:, :], in_=sums_dve)
            nc.sync.dma_start(out=sums_act_d[:, :], in_=sums_act)
    nc.compile()
    return nc


def _build_k2(SL, repeat=1):
    W = HALVES * SL
    nc = bacc.Bacc("TRN2", target_bir_lowering=False, debug=False,
                   num_devices=NCORES)
    ins = {}
    for name in ("sx", "sy", "sz"):
        ins[name] = nc.declare_dram_parameter(name, [ROWS, W], F32,
                                              isOutput=False)
    scal_d = nc.declare_dram_parameter("scal", [ROWS, 24], F32, isOutput=False)
    outs = {}
    for name in ("ax", "ay", "az"):
        outs[name] = nc.declare_dram_parameter(name, [ROWS, W], F32,
                                               isOutput=True)

    with tile.TileContext(nc) as tc:
        with (
            tc.tile_pool(name="data", bufs=2) as data,
            tc.tile_pool(name="tmp", bufs=3) as tmp,
            tc.tile_pool(name="small", bufs=1) as small,
        ):
            scal = small.tile([ROWS, 24], F32, tag="scal")
            nc.sync.dma_start(out=scal, in_=scal_d[:, :])

            for _ in range(repeat):
                for h in range(HALVES):
                    t_in = {}
                    for name in ("sx", "sy", "sz"):
                        t_in[name] = data.tile([ROWS, SL], F32,
                                               name=f"{name}{h}",
                                               tag=f"in_{name}")
                        nc.sync.dma_start(out=t_in[name],
                                          in_=ins[name][:, h * SL:(h + 1) * SL])

                    for i, on in enumerate(("ax", "ay", "az")):
                        c = h * 12 + i * 4     # cols: R_i0, R_i1, R_i2, t_i
                        # a = R_i0*sx + t_i      (ScalarE affine)
                        a = tmp.tile([ROWS, SL], F32, tag="a")
                        nc.scalar.activation(
                            out=a,
                            in_=t_in["sx"],
                            func=mybir.ActivationFunctionType.Identity,
                            bias=scal[:, c + 3:c + 4],
                            scale=scal[:, c:c + 1],
                        )
                        # b = R_i1*sy + a        (VectorE FMA)
                        b = tmp.tile([ROWS, SL], F32, tag="b")
                        nc.vector.scalar_tensor_tensor(
                            out=b, in0=t_in["sy"],
                            scalar=scal[:, c + 1:c + 2],
                            in1=a, op0=mybir.AluOpType.mult,
                            op1=mybir.AluOpType.add,
                        )
                        # out = R_i2*sz + b      (VectorE FMA)
                        o = tmp.tile([ROWS, SL], F32, tag="o")
                        nc.vector.scalar_tensor_tensor(
                            out=o, in0=t_in["sz"],
                            scalar=scal[:, c + 2:c + 3],
                            in1=b, op0=mybir.AluOpType.mult,
                            op1=mybir.AluOpType.add,
                        )
                        nc.sync.dma_start(
                            out=outs[on][:, h * SL:(h + 1) * SL], in_=o)
    nc.compile()
    return nc


def _get_programs(SL):
    key = ("progs", SL)
    if key not in _cache:
        _cache[key] = (_build_k1(SL), _build_k2(SL))
    return _cache[key]


def kernel(src_points, tgt_points, batch_indices):
    src = np.ascontiguousarray(np.asarray(src_points, dtype=np.float32))
    tgt = np.ascontiguousarray(np.asarray(tgt_points, dtype=np.float32))
    bidx = np.asarray(batch_indices)
    n_pts = src.shape[0]
    assert src.shape == (n_pts, 3) and tgt.shape == (n_pts, 3)

    bounds = np.searchsorted(bidx, np.arange(B + 1), side="left")
    counts = np.diff(bounds).astype(np.int64)
    max_cnt = int(counts.max())
    SL = max(DEFAULT_SL, -(-max_cnt // 128) * 128)
    W = HALVES * SL

    # ---- pack points segment-per-partition ------------------------------
    # destination flat col for point k of segment g: h*SL + (k - start)
    seg = bidx.astype(np.int64)
    pos = np.arange(n_pts, dtype=np.int64) - bounds[seg]
    core = seg // SPC
    loc = seg % SPC
    half = loc // ROWS
    row = loc % ROWS
    flat = ((core * ROWS + row) * HALVES + half) * SL + pos  # [n_pts]

    srcP = np.zeros((3, NCORES, ROWS, W), np.float32)
    tgtP = np.zeros((3, NCORES, ROWS, W), np.float32)
    # view as [NCORES*ROWS*HALVES*SL] per coord (srcP[c3] is contiguous)
    for c3 in range(3):
        srcP[c3].reshape(-1)[flat] = src[:, c3]
        tgtP[c3].reshape(-1)[flat] = tgt[:, c3]

    nc1, nc2 = _get_programs(SL)

    # ---- kernel 1: per-segment sums -------------------------------------
    in_maps = []
    for c in range(NCORES):
        in_maps.append({
            "sx": srcP[0, c], "sy": srcP[1, c], "sz": srcP[2, c],
            "tx": tgtP[0, c], "ty": tgtP[1, c], "tz": tgtP[2, c],
        })
    r1 = run_bass_kernel_spmd(nc1, in_maps, core_ids=list(range(NCORES)))
    LAST_PROFILE["k1_ns"] = r1.exec_time_ns

    # sums_dve [c][row, h*9+3i+j], sums_act [c][row, h*6+k]
    cross = np.stack([r1.results[c]["sums_dve"] for c in range(NCORES)])
    raw = np.stack([r1.results[c]["sums_act"] for c in range(NCORES)])
    # -> [B, ...]: g = c*SPC + h*ROWS + row
    cross = (cross.reshape(NCORES, ROWS, HALVES, 9)
             .transpose(0, 2, 1, 3).reshape(B, 3, 3).astype(np.float64))
    raw = (raw.reshape(NCORES, ROWS, HALVES, 6)
           .transpose(0, 2, 1, 3).reshape(B, 6).astype(np.float64))
    sum_s, sum_t = raw[:, 0:3], raw[:, 3:6]

    # ---- host: Kabsch via SVD (float64) ---------------------------------
    n = np.maximum(counts.astype(np.float64), 1.0)
    mu_s = sum_s / n[:, None]
    mu_t = sum_t / n[:, None]
    H = cross - n[:, None, None] * (mu_s[:, :, None] * mu_t[:, None, :])
    U, S, Vt = np.linalg.svd(H)
    V = Vt.transpose(0, 2, 1)
    UT = U.transpose(0, 2, 1)
    d = np.sign(np.linalg.det(V @ UT))
    d[d == 0] = 1.0
    scale = np.stack([np.ones_like(d), np.ones_like(d), d], axis=-1)
    R = (V * scale[:, None, :]) @ UT
    t = mu_t - np.einsum("bij,bj->bi", R, mu_s)

    # ---- kernel 2: aligned = R s + t ------------------------------------
    Rf = R.astype(np.float32)
    tf = t.astype(np.float32)
    # scal cols: h*12 + i*4 + (R_i0, R_i1, R_i2, t_i)
    scal = np.zeros((NCORES, ROWS, 24), np.float32)
    Rg = Rf.reshape(NCORES, HALVES, ROWS, 3, 3)
    tg = tf.reshape(NCORES, HALVES, ROWS, 3)
    for h in range(HALVES):
        for i in range(3):
            scal[:, :, h * 12 + i * 4 + 0] = Rg[:, h, :, i, 0]
            scal[:, :, h * 12 + i * 4 + 1] = Rg[:, h, :, i, 1]
            scal[:, :, h * 12 + i * 4 + 2] = Rg[:, h, :, i, 2]
            scal[:, :, h * 12 + i * 4 + 3] = tg[:, h, :, i]

    in_maps2 = []
    for c in range(NCORES):
        in_maps2.append({
            "sx": srcP[0, c], "sy": srcP[1, c], "sz": srcP[2, c],
            "scal": scal[c],
        })
    r2 = run_bass_kernel_spmd(nc2, in_maps2, core_ids=list(range(NCORES)))
    LAST_PROFILE["k2_ns"] = r2.exec_time_ns

    aligned = np.empty((n_pts, 3), np.float32)
    for c3, name in enumerate(("ax", "ay", "az")):
        plane = np.stack([r2.results[c][name] for c in range(NCORES)])
        aligned[:, c3] = plane.reshape(-1)[flat]

    return aligned, (Rf, tf)


# revision 8
# speedup vs baseline: 2.8004x; 2.8004x over previous
"""Batched Procrustes (Kabsch) on 8 Trainium2 NeuronCores.

Strategy (batch-contiguous sharding, fully local per core):
  - 2048 segments -> 8 cores x 256 segments; each core's segments are packed
    segment-per-partition: SBUF tile [128 partitions, SL], one tile per half
    (2 halves of 128 segments), partition p of half h = local segment
    h*128+p, zero-padded to SL points.
  - Kernel 1 (device): per-segment sums of (sx,sy,sz), (tx,ty,tz) and the 9
    cross products s_i*t_j.  Products+reduction fused in one VectorE
    scalar_tensor_tensor (accum_out) pass per product stream; raw sums on
    ScalarE activation accum_out.  15 sums x 256 segments per core.
  - Host: assemble H = cross - n mu_s mu_t^T, 3x3 SVD in float64, Kabsch
    R = V diag(1,1,d) U^T, t = mu_t - R mu_s  (2048 tiny SVDs).
  - Kernel 2 (device): aligned = R[seg] @ src + t[seg] with per-partition
    scalars: ScalarE affine (R_i0*sx + t_i) then two VectorE FMA
    scalar_tensor_tensor chained adds.
  - Host: unpack to original point order; return (aligned, (R, t)).
"""

import numpy as np

import jax
from jax.experimental.shard_map import shard_map
from jax.sharding import Mesh, NamedSharding, PartitionSpec

import concourse.tile as tile
from concourse import bacc, mybir
from concourse import bass2jax
from concourse.bass2jax import _bass_exec_p, partition_id_tensor

N = 4194304
B = 2048
NCORES = 8
SPC = B // NCORES          # segments per core (256)
HALVES = 2
ROWS = 128
DEFAULT_SL = 2304          # per-segment padded length (max count <= SL)

F32 = mybir.dt.float32

_cache = {}
LAST_PROFILE = {}


def _build_k1(SL, repeat=1):
    W = HALVES * SL
    nc = bacc.Bacc("TRN2", target_bir_lowering=False, debug=False,
                   num_devices=NCORES)
    ins = {}
    for name in ("sx", "sy", "sz", "tx", "ty", "tz"):
        ins[name] = nc.declare_dram_parameter(name, [ROWS, W], F32,
                                              isOutput=False)
    sums_dve_d = nc.declare_dram_parameter("sums_dve", [ROWS, 18], F32,
                                           isOutput=True)
    sums_act_d = nc.declare_dram_parameter("sums_act", [ROWS, 12], F32,
                                           isOutput=True)

    with tile.TileContext(nc) as tc:
        with (
            tc.tile_pool(name="data", bufs=2) as data,
            tc.tile_pool(name="small", bufs=1) as small,
        ):
            sums_dve = small.tile([ROWS, 18], F32, tag="sums_dve")
            sums_act = small.tile([ROWS, 12], F32, tag="sums_act")
            trash_dve = small.tile([ROWS, SL], F32, tag="trash_dve")
            trash_act = small.tile([ROWS, SL], F32, tag="trash_act")

            for _ in range(repeat):
                for h in range(HALVES):
                    t_in = {}
                    for name in ("sx", "tx", "sy", "ty", "sz", "tz"):
                        t_in[name] = data.tile([ROWS, SL], F32,
                                               name=f"{name}{h}",
                                               tag=f"in_{name}")
                        nc.sync.dma_start(out=t_in[name],
                                          in_=ins[name][:, h * SL:(h + 1) * SL])
                    # ScalarE: 6 raw sums per half
                    for k, name in enumerate(
                            ("sx", "sy", "sz", "tx", "ty", "tz")):
                        nc.scalar.activation(
                            out=trash_act,
                            in_=t_in[name],
                            func=mybir.ActivationFunctionType.Copy,
                            accum_out=sums_act[:, h * 6 + k: h * 6 + k + 1],
                        )
                    # VectorE: 9 fused product+sum per half
                    for i, sn in enumerate(("sx", "sy", "sz")):
                        for j, tn in enumerate(("tx", "ty", "tz")):
                            col = h * 9 + 3 * i + j
                            nc.vector.scalar_tensor_tensor(
                                out=trash_dve,
                                in0=t_in[sn],
                                scalar=1.0,
                                in1=t_in[tn],
                                op0=mybir.AluOpType.mult,
                                op1=mybir.AluOpType.mult,
                                accum_out=sums_dve[:, col:col + 1],
                            )

            nc.sync.dma_start(out=sums_dve_d[:, :], in_=sums_dve)
            nc.sync.dma_start(out=sums_act_d[:, :], in_=sums_act)
    nc.compile()
    return nc


def _build_k2(SL, repeat=1):
    W = HALVES * SL
    nc = bacc.Bacc("TRN2", target_bir_lowering=False, debug=False,
                   num_devices=NCORES)
    ins = {}
    for name in ("sx", "sy", "sz"):
        ins[name] = nc.declare_dram_parameter(name, [ROWS, W], F32,
                                              isOutput=False)
    scal_d = nc.declare_dram_parameter("scal", [ROWS, 24], F32, isOutput=False)
    outs = {}
    for name in ("ax", "ay", "az"):
        outs[name] = nc.declare_dram_parameter(name, [ROWS, W], F32,
                                               isOutput=True)

    with tile.TileContext(nc) as tc:
        with (
            tc.tile_pool(name="data", bufs=2) as data,
            tc.tile_pool(name="tmp", bufs=3) as tmp,
            tc.tile_pool(name="small", bufs=1) as small,
        ):
            scal = small.tile([ROWS, 24], F32, tag="scal")
            nc.sync.dma_start(out=scal, in_=scal_d[:, :])

            for _ in range(repeat):
                for h in range(HALVES):
                    t_in = {}
                    for name in ("sx", "sy", "sz"):
                        t_in[name] = data.tile([ROWS, SL], F32,
                                               name=f"{name}{h}",
                                               tag=f"in_{name}")
                        nc.sync.dma_start(out=t_in[name],
                                          in_=ins[name][:, h * SL:(h + 1) * SL])

                    for i, on in enumerate(("ax", "ay", "az")):
                        c = h * 12 + i * 4     # cols: R_i0, R_i1, R_i2, t_i
                        # a = R_i0*sx + t_i      (ScalarE affine)
                        a = tmp.tile([ROWS, SL], F32, tag="a")
                        nc.scalar.activation(
                            out=a,
                            in_=t_in["sx"],
                            func=mybir.ActivationFunctionType.Identity,
                            bias=scal[:, c + 3:c + 4],
                            scale=scal[:, c:c + 1],
                        )
                        # b = R_i1*sy + a        (VectorE FMA)
                        b = tmp.tile([ROWS, SL], F32, tag="b")
                        nc.vector.scalar_tensor_tensor(
                            out=b, in0=t_in["sy"],
                            scalar=scal[:, c + 1:c + 2],
                            in1=a, op0=mybir.AluOpType.mult,
                            op1=mybir.AluOpType.add,
                        )
                        # out = R_i2*sz + b      (VectorE FMA)
                        o = tmp.tile([ROWS, SL], F32, tag="o")
                        nc.vector.scalar_tensor_tensor(
                            out=o, in0=t_in["sz"],
                            scalar=scal[:, c + 2:c + 3],
                            in1=b, op0=mybir.AluOpType.mult,
                            op1=mybir.AluOpType.add,
                        )
                        nc.sync.dma_start(
                            out=outs[on][:, h * SL:(h + 1) * SL], in_=o)
    nc.compile()
    return nc


class Runner:
    """Reusable sharded jit for a compiled Bass program (multi-core SPMD,
    outputs donated from device-created zero buffers)."""

    def __init__(self, nc, n_cores=NCORES):
        bass2jax.install_neuronx_cc_hook()
        self.n_cores = n_cores
        pname = nc.partition_id_tensor.name if nc.partition_id_tensor else None
        in_names, out_names, out_avals = [], [], []
        for alloc in nc.m.functions[0].allocations:
            if not isinstance(alloc, mybir.MemoryLocationSet):
                continue
            name = alloc.memorylocations[0].name
            if alloc.kind == "ExternalInput":
                if name != pname:
                    in_names.append(name)
            elif alloc.kind == "ExternalOutput":
                out_names.append(name)
                out_avals.append(
                    jax.core.ShapedArray(tuple(alloc.tensor_shape),
                                         mybir.dt.np(alloc.dtype)))
        self.in_names, self.out_names, self.out_avals = (
            in_names, out_names, out_avals)
        n_params = len(in_names)
        all_in = in_names + out_names + ([pname] if pname else [])
        donate = tuple(range(n_params, n_params + len(out_names)))

        def _body(*args):
            operands = list(args)
            if pname is not None:
                operands.append(partition_id_tensor())
            return tuple(_bass_exec_p.bind(
                *operands,
                out_avals=tuple(out_avals),
                in_names=tuple(all_in),
                out_names=tuple(out_names),
                lowering_input_output_aliases=(),
                sim_require_finite=True,
                sim_require_nnan=True,
                nc=nc,
            ))

        devices = jax.devices()[:n_cores]
        self.mesh = Mesh(np.asarray(devices), ("core",))
        self.sharding = NamedSharding(self.mesh, PartitionSpec("core"))
        n_out = len(out_names)
        self.fn = jax.jit(
            shard_map(_body, mesh=self.mesh,
                      in_specs=(PartitionSpec("core"),) * (n_params + n_out),
                      out_specs=(PartitionSpec("core"),) * n_out,
                      check_rep=False),
            donate_argnums=donate, keep_unused=True)
        self._zeros_fn = jax.jit(
            lambda: tuple(
                jax.numpy.zeros((n_cores * av.shape[0], *av.shape[1:]),
                                av.dtype)
                for av in out_avals),
            out_shardings=tuple(self.sharding for _ in out_avals))

    def put_inputs(self, in_maps):
        return [
            jax.device_put(
                np.concatenate([np.asarray(in_maps[c][n])
                                for c in range(self.n_cores)], axis=0),
                self.sharding)
            for n in self.in_names
        ]

    def run_device(self, dev_args):
        """dev_args: list from put_inputs. Returns list of jax arrays."""
        return self.fn(*dev_args, *self._zeros_fn())

    def run(self, in_maps):
        outs = self.run_device(self.put_inputs(in_maps))
        res = []
        for c in range(self.n_cores):
            res.append({
                name: np.asarray(outs[i])[c * self.out_avals[i].shape[0]:
                                          (c + 1) * self.out_avals[i].shape[0]]
                for i, name in enumerate(self.out_names)
            })
        return res


def _get_runners(SL):
    key = ("runners", SL)
    if key not in _cache:
        _cache[key] = (Runner(_build_k1(SL)), Runner(_build_k2(SL)))
    return _cache[key]


def kernel(src_points, tgt_points, batch_indices):
    src = np.ascontiguousarray(np.asarray(src_points, dtype=np.float32))
    tgt = np.ascontiguousarray(np.asarray(tgt_points, dtype=np.float32))
    bidx = np.asarray(batch_indices)
    n_pts = src.shape[0]
    assert src.shape == (n_pts, 3) and tgt.shape == (n_pts, 3)

    bounds = np.searchsorted(bidx, np.arange(B + 1), side="left")
    counts = np.diff(bounds).astype(np.int64)
    max_cnt = int(counts.max())
    SL = max(DEFAULT_SL, -(-max_cnt // 128) * 128)
    W = HALVES * SL

    # ---- pack points segment-per-partition ------------------------------
    seg = bidx.astype(np.int64)
    pos = np.arange(n_pts, dtype=np.int64) - bounds[seg]
    core = seg // SPC
    loc = seg % SPC
    half = loc // ROWS
    row = loc % ROWS
    flat = ((core * ROWS + row) * HALVES + half) * SL + pos  # [n_pts]

    srcP = np.zeros((3, NCORES, ROWS, W), np.float32)
    tgtP = np.zeros((3, NCORES, ROWS, W), np.float32)
    for c3 in range(3):
        srcP[c3].reshape(-1)[flat] = src[:, c3]
        tgtP[c3].reshape(-1)[flat] = tgt[:, c3]

    run1, run2 = _get_runners(SL)

    # ---- kernel 1: per-segment sums -------------------------------------
    in_maps = [{"sx": srcP[0, c], "sy": srcP[1, c], "sz": srcP[2, c],
                "tx": tgtP[0, c], "ty": tgtP[1, c], "tz": tgtP[2, c]}
               for c in range(NCORES)]
    res1 = run1.run(in_maps)

    cross = np.stack([res1[c]["sums_dve"] for c in range(NCORES)])
    raw = np.stack([res1[c]["sums_act"] for c in range(NCORES)])
    cross = (cross.reshape(NCORES, ROWS, HALVES, 9)
             .transpose(0, 2, 1, 3).reshape(B, 3, 3).astype(np.float64))
    raw = (raw.reshape(NCORES, ROWS, HALVES, 6)
           .transpose(0, 2, 1, 3).reshape(B, 6).astype(np.float64))
    sum_s, sum_t = raw[:, 0:3], raw[:, 3:6]

    # ---- host: Kabsch via SVD (float64) ---------------------------------
    n = np.maximum(counts.astype(np.float64), 1.0)
    mu_s = sum_s / n[:, None]
    mu_t = sum_t / n[:, None]
    H = cross - n[:, None, None] * (mu_s[:, :, None] * mu_t[:, None, :])
    U, S, Vt = np.linalg.svd(H)
    V = Vt.transpose(0, 2, 1)
    UT = U.transpose(0, 2, 1)
    d = np.sign(np.linalg.det(V @ UT))
    d[d == 0] = 1.0
    scale = np.stack([np.ones_like(d), np.ones_like(d), d], axis=-1)
    R = (V * scale[:, None, :]) @ UT
    t = mu_t - np.einsum("bij,bj->bi", R, mu_s)

    # ---- kernel 2: aligned = R s + t ------------------------------------
    Rf = R.astype(np.float32)
    tf = t.astype(np.float32)
    scal = np.zeros((NCORES, ROWS, 24), np.float32)
    Rg = Rf.reshape(NCORES, HALVES, ROWS, 3, 3)
    tg = tf.reshape(NCORES, HALVES, ROWS, 3)
    for h in range(HALVES):
        for i in range(3):
            scal[:, :, h * 12 + i * 4 + 0] = Rg[:, h, :, i, 0]
            scal[:, :, h * 12 + i * 4 + 1] = Rg[:, h, :, i, 1]
            scal[:, :, h * 12 + i * 4 + 2] = Rg[:, h, :, i, 2]
            scal[:, :, h * 12 + i * 4 + 3] = tg[:, h, :, i]

    in_maps2 = [{"sx": srcP[0, c], "sy": srcP[1, c], "sz": srcP[2, c],
                 "scal": scal[c]} for c in range(NCORES)]
    res2 = run2.run(in_maps2)

    aligned = np.empty((n_pts, 3), np.float32)
    for c3, name in enumerate(("ax", "ay", "az")):
        plane = np.stack([res2[c][name] for c in range(NCORES)])
        aligned[:, c3] = plane.reshape(-1)[flat]

    return aligned, (Rf, tf)
